# revision 37
# baseline (speedup 1.0000x reference)
"""GAT 3-layer network on 8 Trainium2 NeuronCores.

Strategy (graph/node parallel):
  - Nodes partitioned contiguously across 8 cores (2500/core, padded to 2560).
  - Edges assigned to the core owning their dst node, dst-sorted, grouped by
    128-dst blocks; per-block chunk counts equalized across cores so one SPMD
    program serves all cores.
  - Per layer: each core transforms its own nodes with a fused weight
    [W | W@blockdiag(a_src) | W@blockdiag(a_dst)] producing rows
    [H | alpha_src | alpha_dst | pad] of a node table; the table is
    AllGathered so every core holds all 20480 rows in HBM.
  - Message passing: dma_gather fetches per-edge source rows (features +
    alpha_src in one fetch) plus a small per-edge fetch of alpha_dst by dst.
    ex = exp(leakyrelu(a_s + a_d)) is computed per edge; aggregation is done
    with PE matmuls PSUM[dst, :] += S^T @ [ex*G | ex] where S is a 0/1
    indicator built by iota/is_equal; the softmax denominator rides along as
    extra columns and the division is applied post-aggregation (algebraically
    identical to the reference).
  - Layer 1 skips the AllGather: x is replicated, so every core builds the
    full layer-1 table locally.
  - Output: per-core h3 slices + per-core pooling partial sums (PE matmul
    against a graph-indicator); host concatenates/combines.
"""

import os
import numpy as np
from contextlib import ExitStack

import concourse.bass as bass
import concourse.mybir as mybir
import concourse.tile as tile
from concourse import bacc
from concourse.masks import make_identity

F32 = mybir.dt.float32
I16 = mybir.dt.int16
AF = mybir.ActivationFunctionType
ALU = mybir.AluOpType

NCORES = 8
NEG_SLOPE = 0.2


def _round_up(a, b):
    return (a + b - 1) // b * b


def _blockdiag(a):
    # a: [H, C] -> [H*C, H]
    H, C = a.shape
    out = np.zeros((H * C, H), np.float32)
    for h in range(H):
        out[h * C:(h + 1) * C, h] = a[h]
    return out


def _wrap16(vals, npart=128):
    # vals: [E] -> [128, E//16] with vals[i] at [i%16, i//16], replicated in
    # every 16-partition group (the 8 gpsimd cores each read their group).
    E = len(vals)
    assert E % 16 == 0
    arr = np.zeros((npart, E // 16), vals.dtype)
    base = vals.reshape(E // 16, 16).T  # [16, E//16]
    for g in range(npart // 16):
        arr[16 * g:16 * (g + 1), :] = base
    return arr


def _prep(x, edge_index, batch, Ws, a_srcs, a_dsts, bs):
    """Host-side preprocessing. Returns dims dict + per-core input maps."""
    N, FIN = x.shape
    G = int(batch.max()) + 1
    NPC = (N + NCORES - 1) // NCORES
    PADN = _round_up(NPC, 128)
    R = NCORES * PADN
    NBLK = PADN // 128

    src = np.concatenate([edge_index[0], np.arange(N, dtype=np.int64)])
    dst = np.concatenate([edge_index[1], np.arange(N, dtype=np.int64)])

    glob2row = (src // NPC) * PADN + (src % NPC)  # table row of each edge src

    # per-core, per-block edge lists
    core_blk_edges = []  # [core][block] -> (srcrow array, dstoff array, dstg array)
    blk_counts = np.zeros((NCORES, NBLK), np.int64)
    for c in range(NCORES):
        lo, hi = c * NPC, min(N, (c + 1) * NPC)
        sel = (dst >= lo) & (dst < hi)
        es = glob2row[sel]
        ed = (dst[sel] - lo).astype(np.int64)
        order = np.argsort(ed, kind="stable")
        es, ed = es[order], ed[order]
        blocks = []
        bid = ed // 128
        for b in range(NBLK):
            m = bid == b
            blocks.append((es[m], ed[m]))
            blk_counts[c, b] = m.sum()
        core_blk_edges.append(blocks)

    nb = [max(1, int(np.ceil(blk_counts[:, b].max() / 128))) for b in range(NBLK)]
    NCH = int(sum(nb))
    EPAD = 128 * NCH

    per_core = []
    for c in range(NCORES):
        srcg = np.zeros(EPAD, np.int16)
        dstg = np.zeros(EPAD, np.int16)
        dstoff = np.full(EPAD, -1.0, np.float32)
        pos = 0
        for b in range(NBLK):
            es, ed = core_blk_edges[c][b]
            k = len(es)
            cap = nb[b] * 128
            srcg[pos:pos + k] = es.astype(np.int16)
            dstg[pos:pos + k] = (c * PADN + ed).astype(np.int16)
            dstoff[pos:pos + k] = (ed - 128 * b).astype(np.float32)
            # padding: srcg stays 0 (valid row), dstg -> own base row, dstoff -1
            dstg[pos + k:pos + cap] = np.int16(c * PADN)
            pos += cap
        assert pos == EPAD
        srcg16 = _wrap16(srcg)
        dstg16 = _wrap16(dstg)
        dstoffL = np.full((128, NCH), -1.0, np.float32)
        dstoffL[:, :] = dstoff.reshape(NCH, 128).T
        # batchoff: graph id of each own node, -1 for padding
        boff = np.full((128, NBLK), -1.0, np.float32)
        lo, hi = c * NPC, min(N, (c + 1) * NPC)
        ids = np.full(PADN, -1.0, np.float32)
        ids[: hi - lo] = batch[lo:hi].astype(np.float32)
        boff[:, :] = ids.reshape(NBLK, 128).T
        per_core.append(dict(srcg16=srcg16, dstg16=dstg16, dstoff=dstoffL,
                             batchoff=boff))

    # fused weights per layer; rows [H | a_src | a_dst | pad]
    bf16 = os.environ.get("GAT_BF16", "0") == "1"
    gran = 128 if bf16 else 64  # elements per 256B (table dtype granule)
    FHs = [Ws[0].shape[1], Ws[1].shape[1], Ws[2].shape[1]]  # 256,256,128
    nas = [a.shape[0] for a in a_srcs]  # 4,4,1
    ROWs = [_round_up(FHs[i] + 2 * nas[i], gran) for i in range(3)]
    wcats = []
    for i in range(3):
        W, asr, ads = Ws[i], a_srcs[i], a_dsts[i]
        wc = np.zeros((W.shape[0], ROWs[i]), np.float32)
        wc[:, :FHs[i]] = W
        wc[:, FHs[i]:FHs[i] + nas[i]] = W @ _blockdiag(asr)
        wc[:, FHs[i] + nas[i]:FHs[i] + 2 * nas[i]] = W @ _blockdiag(ads)
        wcats.append(wc)
    # pack multi-K-tile wcats as [128, nk*ROW]
    def packw(wc):
        K, ROW = wc.shape
        nk = K // 128
        return np.concatenate([wc[128 * k:128 * (k + 1), :] for k in range(nk)],
                              axis=1).astype(np.float32)
    if bf16:
        import ml_dtypes
        npdt = ml_dtypes.bfloat16
    else:
        npdt = np.float32
    wcat1 = packw(wcats[0]).astype(npdt)
    wcat2 = packw(wcats[1]).astype(npdt)
    wcat3 = packw(wcats[2]).astype(npdt)

    bias = np.zeros((128, FHs[0] + FHs[1] + FHs[2]), np.float32)
    bias[:, :] = np.concatenate([bs[0], bs[1], bs[2]])[None, :]

    iota = np.tile(np.arange(128, dtype=np.float32), (128, 1))

    xrep = np.zeros((R, FIN), np.float32)
    for c in range(NCORES):
        lo, hi = c * NPC, min(N, (c + 1) * NPC)
        xrep[c * PADN:c * PADN + hi - lo] = x[lo:hi]
    xT = np.ascontiguousarray(xrep.T).astype(npdt)  # [FIN, R]

    dims = dict(N=N, FIN=FIN, G=G, NPC=NPC, PADN=PADN, R=R, NBLK=NBLK,
                nb=nb, NCH=NCH, EPAD=EPAD, FHs=FHs, nas=nas, ROWs=ROWs,
                bf16=bf16)
    shared = dict(xT=xT, wcat1=wcat1, wcat2=wcat2, wcat3=wcat3, bias=bias,
                  iota=iota)
    in_maps = []
    for c in range(NCORES):
        m = dict(shared)
        m.update(per_core[c])
        in_maps.append(m)
    return dims, in_maps


def _build(dims):
    """Build the SPMD bass program for one core."""
    N = dims["N"]; FIN = dims["FIN"]; G = dims["G"]
    PADN = dims["PADN"]; R = dims["R"]; NBLK = dims["NBLK"]
    nb = dims["nb"]; NCH = dims["NCH"]; EPAD = dims["EPAD"]
    FHs = dims["FHs"]; nas = dims["nas"]; ROWs = dims["ROWs"]
    BSUM = FHs[0] + FHs[1] + FHs[2]
    TD = mybir.dt.bfloat16 if dims["bf16"] else F32
    SUBW = 128 if dims["bf16"] else 64  # alpha sub-gather width (256B)

    nc = bacc.Bacc("TRN2", target_bir_lowering=False, debug=False,
                   enable_asserts=False, num_devices=NCORES,
                   num_swdge_queues=1)

    # --- DRAM I/O ---
    d_xT = nc.dram_tensor("xT", [FIN, R], TD, kind="ExternalInput").ap()
    d_w1 = nc.dram_tensor("wcat1", [128, (FIN // 128) * ROWs[0]], TD,
                          kind="ExternalInput").ap()
    d_w2 = nc.dram_tensor("wcat2", [128, (FHs[0] // 128) * ROWs[1]], TD,
                          kind="ExternalInput").ap()
    d_w3 = nc.dram_tensor("wcat3", [128, (FHs[1] // 128) * ROWs[2]], TD,
                          kind="ExternalInput").ap()
    d_bias = nc.dram_tensor("bias", [128, BSUM], F32, kind="ExternalInput").ap()
    d_iota = nc.dram_tensor("iota", [128, 128], F32, kind="ExternalInput").ap()
    d_dstoff = nc.dram_tensor("dstoff", [128, NCH], F32, kind="ExternalInput").ap()
    d_boff = nc.dram_tensor("batchoff", [128, NBLK], F32, kind="ExternalInput").ap()
    d_srcg = nc.dram_tensor("srcg16", [128, EPAD // 16], I16,
                            kind="ExternalInput").ap()
    d_dstg = nc.dram_tensor("dstg16", [128, EPAD // 16], I16,
                            kind="ExternalInput").ap()

    d_h3 = nc.dram_tensor("h3", [PADN, FHs[2]], F32, kind="ExternalOutput").ap()
    d_pool = nc.dram_tensor("pool", [_round_up(G, 64), FHs[2]], F32,
                            kind="ExternalOutput").ap()

    # tables
    T = [nc.dram_tensor("T1", [R, ROWs[0]], TD, kind="Internal").ap(),
         nc.dram_tensor("T2", [R, ROWs[1]], TD, kind="Internal").ap(),
         nc.dram_tensor("T3", [R, ROWs[2]], TD, kind="Internal").ap()]
    Sh = [None,
          nc.dram_tensor("Sh2", [PADN, ROWs[1]], TD, kind="Internal").ap(),
          nc.dram_tensor("Sh3", [PADN, ROWs[2]], TD, kind="Internal").ap()]

    rg = [list(range(NCORES))]

    with tile.TileContext(nc) as tc, ExitStack() as ctx:
        cpool = ctx.enter_context(tc.tile_pool(name="const", bufs=1))
        gpool = ctx.enter_context(tc.tile_pool(name="gath", bufs=2))
        apool = ctx.enter_context(tc.tile_pool(name="alph", bufs=2))
        spool = ctx.enter_context(tc.tile_pool(name="smat", bufs=2))
        epool = ctx.enter_context(tc.tile_pool(name="exg", bufs=2))
        upool = ctx.enter_context(tc.tile_pool(name="utile", bufs=3))
        ppool = ctx.enter_context(tc.tile_pool(name="post", bufs=3))
        zpool = ctx.enter_context(tc.tile_pool(name="ztile", bufs=2))
        wpool = ctx.enter_context(tc.tile_pool(name="tx", bufs=3))
        psum_a = ctx.enter_context(tc.tile_pool(name="ps_agg", bufs=2, space="PSUM"))
        psum_t = ctx.enter_context(tc.tile_pool(name="ps_tx", bufs=2, space="PSUM"))
        psum_tr = ctx.enter_context(tc.tile_pool(name="ps_tr", bufs=2, space="PSUM"))
        psum_p = ctx.enter_context(tc.tile_pool(name="ps_pool", bufs=1, space="PSUM"))

        # --- constants ---
        ident = cpool.tile([128, 128], F32)
        make_identity(nc, ident[:])
        c_w1 = cpool.tile([128, (FIN // 128) * ROWs[0]], TD)
        nc.sync.dma_start(out=c_w1[:], in_=d_w1)
        c_w2 = cpool.tile([128, 2 * ROWs[1]], TD)
        nc.sync.dma_start(out=c_w2[:], in_=d_w2)
        c_w3 = cpool.tile([128, 2 * ROWs[2]], TD)
        nc.sync.dma_start(out=c_w3[:], in_=d_w3)
        c_bias = cpool.tile([128, BSUM], F32)
        nc.sync.dma_start(out=c_bias[:], in_=d_bias)
        c_iota = cpool.tile([128, 128], F32)
        nc.sync.dma_start(out=c_iota[:], in_=d_iota)
        c_dstoff = cpool.tile([128, NCH], F32)
        nc.sync.dma_start(out=c_dstoff[:], in_=d_dstoff)
        c_boff = cpool.tile([128, NBLK], F32)
        nc.sync.dma_start(out=c_boff[:], in_=d_boff)
        c_srcg = cpool.tile([128, EPAD // 16], I16)
        nc.sync.dma_start(out=c_srcg[:], in_=d_srcg)
        c_dstg = cpool.tile([128, EPAD // 16], I16)
        nc.sync.dma_start(out=c_dstg[:], in_=d_dstg)

        # --- layer 1 transform, replicated over all R rows ---
        for t in range(R // 128):
            xt = wpool.tile([128, 128], TD, tag="xt")
            nc.sync.dma_start(out=xt[:], in_=d_xT[:, 128 * t:128 * (t + 1)])
            ps = psum_t.tile([128, ROWs[0]], F32, tag="ptx")
            nc.tensor.matmul(out=ps[:], lhsT=xt[:], rhs=c_w1[:, :ROWs[0]],
                             start=True, stop=True)
            hb = wpool.tile([128, ROWs[0]], TD, tag="hout")
            nc.vector.tensor_copy(out=hb[:], in_=ps[:])
            nc.sync.dma_start(out=T[0][128 * t:128 * (t + 1), :], in_=hb[:])

        # --- per layer ---
        pool_ps = psum_p.tile([64, FHs[2]], F32)
        for L in range(3):
            FH, na, ROW = FHs[L], nas[L], ROWs[L]
            NR = FH + na
            inner = FH // na
            tbl = T[L]
            ch0 = 0  # running chunk offset
            for b in range(NBLK):
                nbb = nb[b]
                ps = psum_a.tile([128, NR], F32, tag="agg")
                gmax = int(os.environ.get("GAT_GMAX", "10"))
                halves = [(o, min(gmax, nbb - o)) for o in range(0, nbb, gmax)]
                for (goff, nhb) in halves:
                    cc0 = ch0 + goff
                    e0 = 128 * cc0
                    n = 128 * nhb
                    # gather [H | a_s | a_d | pad] rows by src
                    Gt = gpool.tile([128, nhb * ROW], TD, tag="G")
                    nc.gpsimd.dma_gather(
                        out_ap=Gt[:].rearrange("p (c r) -> p c r", r=ROW),
                        in_ap=tbl[:, :],
                        idxs_ap=c_srcg[:, e0 // 16:(e0 + n) // 16],
                        num_idxs=n, num_idxs_reg=n, elem_size=ROW,
                        single_packet=(n <= 960), queue_num=0)
                    # gather alpha sub-rows by dst (256B tail of the row)
                    At = apool.tile([128, nhb * SUBW], TD, tag="A")
                    nc.gpsimd.dma_gather(
                        out_ap=At[:].rearrange("p (c r) -> p c r", r=SUBW),
                        in_ap=tbl[:, FH:FH + SUBW],
                        idxs_ap=c_dstg[:, e0 // 16:(e0 + n) // 16],
                        num_idxs=n, num_idxs_reg=n, elem_size=SUBW,
                        elem_step=ROW, single_packet=(n <= 960), queue_num=0)
                    G3 = Gt[:].rearrange("p (c r) -> p c r", r=ROW)
                    A3 = At[:].rearrange("p (c r) -> p c r", r=SUBW)
                    # u = a_s[src] + a_d[dst]
                    ut = upool.tile([128, nhb * na], F32, tag="u")
                    u3 = ut[:].rearrange("p (c a) -> p c a", a=na)
                    nc.vector.tensor_tensor(out=u3, in0=G3[:, :, FH:FH + na],
                                            in1=A3[:, :, na:2 * na], op=ALU.add)
                    # lrelu(u) = max(u, 0.2*u)
                    u2 = upool.tile([128, nhb * na], F32, tag="u2")
                    nc.vector.tensor_scalar_mul(u2[:], ut[:], NEG_SLOPE)
                    nc.vector.tensor_tensor(out=ut[:], in0=ut[:], in1=u2[:],
                                            op=ALU.max)
                    # exG tile holds [ex*G | ex]
                    Et = epool.tile([128, nhb * NR], TD, tag="E")
                    E3 = Et[:].rearrange("p (c r) -> p c r", r=NR)
                    nc.scalar.activation(out=E3[:, :, FH:FH + na], in_=u3,
                                         func=AF.Exp)
                    ex_b = (E3[:, :, FH:FH + na]
                            .rearrange("p c (a one) -> p c a one", one=1)
                            .to_broadcast([128, nhb, na, inner]))
                    nc.vector.tensor_tensor(
                        out=E3[:, :, 0:FH].rearrange("p c (a i) -> p c a i",
                                                     i=inner),
                        in0=G3[:, :, 0:FH].rearrange("p c (a i) -> p c a i",
                                                     i=inner),
                        in1=ex_b, op=ALU.mult)
                    # indicator S
                    St = spool.tile([128, nhb * 128], TD, tag="S")
                    S3 = St[:].rearrange("p (c j) -> p c j", j=128)
                    nc.vector.tensor_tensor(
                        out=S3,
                        in0=c_dstoff[:, cc0:cc0 + nhb]
                        .rearrange("p (c one) -> p c one", one=1)
                        .to_broadcast([128, nhb, 128]),
                        in1=c_iota[:].rearrange("p (o j) -> p o j", o=1)
                        .to_broadcast([128, nhb, 128]),
                        op=ALU.is_equal)
                    for cc in range(nhb):
                        gc = goff + cc
                        nc.tensor.matmul(out=ps[:], lhsT=S3[:, cc, :],
                                         rhs=E3[:, cc, :],
                                         start=(gc == 0), stop=(gc == nbb - 1),
                                         skip_group_check=True)
                # --- post-process block b ---
                den = ppool.tile([128, na], F32, tag="den")
                nc.vector.tensor_scalar_add(den[:], ps[:, FH:FH + na], 1e-16)
                rec = ppool.tile([128, na], F32, tag="rec")
                nc.vector.reciprocal(rec[:], den[:])
                Y = ppool.tile([128, FH], F32, tag="Y")
                nc.vector.tensor_tensor(
                    out=Y[:].rearrange("p (a i) -> p a i", i=inner),
                    in0=ps[:, 0:FH].rearrange("p (a i) -> p a i", i=inner),
                    in1=rec[:].rearrange("p (a one) -> p a one", one=1)
                    .to_broadcast([128, na, inner]),
                    op=ALU.mult)
                boff = sum(FHs[:L])
                nc.vector.tensor_add(Y[:], Y[:], c_bias[:, boff:boff + FH])
                if L < 2:
                    # elu(Y) = max(Y,0) + exp(min(Y,0)) - 1
                    rt = ppool.tile([128, FH], F32, tag="rt")
                    nc.vector.tensor_scalar_max(rt[:], Y[:], 0.0)
                    mt = ppool.tile([128, FH], F32, tag="mt")
                    nc.vector.tensor_scalar_min(mt[:], Y[:], 0.0)
                    nc.scalar.activation(out=mt[:], in_=mt[:], func=AF.Exp)
                    Z = zpool.tile([128, FH], F32, tag="Z")
                    nc.vector.tensor_add(Z[:], rt[:], mt[:])
                    nc.vector.tensor_scalar_add(Z[:], Z[:], -1.0)
                    # transform to next layer: H = Z @ Wcat_{L+1}
                    wn = c_w2 if L == 0 else c_w3
                    ROWn = ROWs[L + 1]
                    pst = psum_t.tile([128, ROWn], F32, tag="ptx")
                    for h in range(FH // 128):
                        ptr = psum_tr.tile([128, 128], F32)
                        nc.tensor.transpose(out=ptr[:],
                                            in_=Z[:, 128 * h:128 * (h + 1)],
                                            identity=ident[:])
                        zt = zpool.tile([128, 128], TD, tag="zt")
                        nc.vector.tensor_copy(out=zt[:], in_=ptr[:])
                        nc.tensor.matmul(out=pst[:], lhsT=zt[:],
                                         rhs=wn[:, ROWn * h:ROWn * (h + 1)],
                                         start=(h == 0), stop=(h == FH // 128 - 1),
                                         skip_group_check=True)
                    hn = wpool.tile([128, ROWn], TD, tag="hnext")
                    nc.vector.tensor_copy(out=hn[:], in_=pst[:])
                    nc.sync.dma_start(
                        out=Sh[L + 1][128 * b:128 * (b + 1), :], in_=hn[:])
                else:
                    # final layer: h3 out + pooling partials
                    nc.sync.dma_start(out=d_h3[128 * b:128 * (b + 1), :],
                                      in_=Y[:])
                    Sp = ppool.tile([128, 64], F32, tag="Sp")
                    nc.vector.tensor_tensor(
                        out=Sp[:].rearrange("p (c j) -> p c j", c=1),
                        in0=c_boff[:, b:b + 1]
                        .rearrange("p (c one) -> p c one", one=1)
                        .to_broadcast([128, 1, 64]),
                        in1=c_iota[:, :64].rearrange("p (o j) -> p o j", o=1)
                        .to_broadcast([128, 1, 64]),
                        op=ALU.is_equal)
                    nc.tensor.matmul(out=pool_ps[:], lhsT=Sp[:], rhs=Y[:],
                                     start=(b == 0), stop=(b == NBLK - 1),
                                     skip_group_check=True)
                ch0 += nbb
            if L < 2:
                if os.environ.get("GAT_NOCOLL") == "1":
                    # debug: local copy instead of AllGather (wrong results)
                    nc.sync.dma_start(out=T[L + 1][:PADN, :], in_=Sh[L + 1])
                else:
                    nc.gpsimd.collective_compute(
                        "AllGather", ALU.bypass, replica_groups=rg,
                        ins=[Sh[L + 1]], outs=[T[L + 1]])
        # pool out
        poolt = ppool.tile([64, FHs[2]], F32, tag="poolout")
        nc.vector.tensor_copy(out=poolt[:], in_=pool_ps[:])
        nc.sync.dma_start(out=d_pool[:64, :], in_=poolt[:])
        zt64 = ppool.tile([64, FHs[2]], F32, tag="zero64")
        nc.vector.memset(zt64[:], 0.0)
        if _round_up(G, 64) > 64:
            nc.sync.dma_start(out=d_pool[64:, :], in_=zt64[:_round_up(G, 64) - 64, :])
    nc.compile()
    return nc


def kernel(**inputs):
    x = np.asarray(inputs["x"], np.float32)
    edge_index = np.asarray(inputs["edge_index"], np.int64)
    batch = np.asarray(inputs["batch"], np.int64)
    Ws = [np.asarray(inputs[k], np.float32) for k in ("W1", "W2", "W3")]
    a_srcs = [np.asarray(inputs[k], np.float32) for k in ("a_src1", "a_src2", "a_src3")]
    a_dsts = [np.asarray(inputs[k], np.float32) for k in ("a_dst1", "a_dst2", "a_dst3")]
    bs = [np.asarray(inputs[k], np.float32) for k in ("b1", "b2", "b3")]

    dims, in_maps = _prep(x, edge_index, batch, Ws, a_srcs, a_dsts, bs)
    nc = _build(dims)

    if os.environ.get("GAT_SIM") == "1":
        results = _run_sim(nc, in_maps)
    elif os.environ.get("GAT_TIME") == "1":
        results = _run_hw_timed(nc, in_maps)
    else:
        from concourse.bass_utils import run_bass_kernel_spmd
        r = run_bass_kernel_spmd(nc, in_maps, core_ids=list(range(NCORES)))
        results = r.results

    N, G, NPC, PADN = dims["N"], dims["G"], dims["NPC"], dims["PADN"]
    h = np.zeros((N, dims["FHs"][2]), np.float32)
    pool_sum = np.zeros((G, dims["FHs"][2]), np.float32)
    for c in range(NCORES):
        lo, hi = c * NPC, min(N, (c + 1) * NPC)
        h[lo:hi] = results[c]["h3"][: hi - lo]
        pool_sum += results[c]["pool"][:G]
    cnts = np.bincount(batch, minlength=G).astype(np.float32)
    graph_embedding = pool_sum / np.maximum(cnts, 1.0)[:, None]
    return graph_embedding, h


def _run_hw_timed(nc, in_maps, reps=3):
    """Like bass2jax.run_bass_via_pjrt but keeps inputs on device and times
    warm re-executions (wall clock, upper bound on device exec time)."""
    import time
    import jax
    import jax.numpy  # noqa
    from jax.sharding import Mesh, PartitionSpec, NamedSharding
    from jax.experimental.shard_map import shard_map
    import concourse.mybir as mybir_
    from concourse import bass2jax
    bass2jax.install_neuronx_cc_hook()

    part_name = nc.partition_id_tensor.name if nc.partition_id_tensor else None
    in_names, out_names, out_avals, zero_outs = [], [], [], []
    for alloc in nc.m.functions[0].allocations:
        if not isinstance(alloc, mybir_.MemoryLocationSet):
            continue
        name = alloc.memorylocations[0].name
        if alloc.kind == "ExternalInput":
            if name != part_name:
                in_names.append(name)
        elif alloc.kind == "ExternalOutput":
            out_names.append(name)
            shape = tuple(alloc.tensor_shape)
            dtype = mybir_.dt.np(alloc.dtype)
            out_avals.append(jax.core.ShapedArray(shape, dtype))
            zero_outs.append(np.zeros(shape, dtype))
    n_params = len(in_names)
    n_outs = len(out_names)
    all_names = list(in_names) + list(out_names)
    if part_name is not None:
        all_names.append(part_name)

    def _body(*args):
        operands = list(args)
        if part_name is not None:
            operands.append(bass2jax.partition_id_tensor())
        outs = bass2jax._bass_exec_p.bind(
            *operands, out_avals=tuple(out_avals), in_names=tuple(all_names),
            out_names=tuple(out_names), lowering_input_output_aliases=(),
            sim_require_finite=True, sim_require_nnan=True, nc=nc)
        return tuple(outs)

    devices = jax.devices()[:NCORES]
    mesh = Mesh(np.asarray(devices), ("core",))
    spec = NamedSharding(mesh, PartitionSpec("core"))
    donate = tuple(range(n_params, n_params + n_outs))
    sharded = jax.jit(
        shard_map(_body, mesh=mesh,
                  in_specs=(PartitionSpec("core"),) * (n_params + n_outs),
                  out_specs=(PartitionSpec("core"),) * n_outs,
                  check_rep=False),
        donate_argnums=donate, keep_unused=True)

    concat_in = [np.concatenate([np.asarray(in_maps[c][nm])
                                 for c in range(NCORES)], axis=0)
                 for nm in in_names]
    dev_in = [jax.device_put(a, spec) for a in concat_in]
    for a in dev_in:
        a.block_until_ready()

    def zeros_dev():
        zs = [jax.device_put(
            np.zeros((NCORES * z.shape[0], *z.shape[1:]), z.dtype), spec)
            for z in zero_outs]
        for z in zs:
            z.block_until_ready()
        return zs

    out_arrs = sharded(*dev_in, *zeros_dev())  # compile + first run
    for o in out_arrs:
        o.block_until_ready()
    times = []
    for _ in range(reps):
        zs = zeros_dev()
        t0 = time.perf_counter()
        out_arrs = sharded(*dev_in, *zs)
        for o in out_arrs:
            o.block_until_ready()
        times.append(time.perf_counter() - t0)
    kernel.last_exec_time_ns = int(min(times) * 1e9)
    kernel.all_times_ns = [int(t * 1e9) for t in times]
    return [
        {nm: np.asarray(out_arrs[i]).reshape(NCORES, *out_avals[i].shape)[c]
         for i, nm in enumerate(out_names)}
        for c in range(NCORES)
    ]


def _run_sim(nc, in_maps):
    from concourse.bass_interp import MultiCoreSim
    sim = MultiCoreSim(nc, num_cores=NCORES, trace=False,
                       require_finite=False, require_nnan=False)
    for c in range(NCORES):
        for k, v in in_maps[c].items():
            sim.cores[c].tensor(k)[:] = v
    sim.simulate(check_with_hw=False)
    out = []
    for c in range(NCORES):
        out.append({k: np.array(sim.cores[c].tensor(k))
                    for k in ("h3", "pool")})
    return out
